# revision 21
# baseline (speedup 1.0000x reference)
"""Trainium2 Bass kernel for causal MQA attention with RMSNorm + pos bias.

Reference:
  xn = rmsnorm(x) * gamma
  q = (xn @ wq) * scale (16 heads x 128);  k = xn @ wk;  v = xn @ wv
  sim = q @ k^T + pos_bias; non-causal entries := 1e-10 (NOT -inf)
  attn = softmax(sim); out = (attn @ v, concat heads) @ wo

Sharding (head-parallel attention, row-parallel norm/k/v):
  Core j owns x rows [256j, 256j+256) for rmsnorm and the k/v
  projections.  xn (bf16 hi/lo) and K/V are AllGathered.  Core j then
  computes q for heads {j, j+8} over ALL 2048 rows from a host-sliced
  wq, so attention is head-sharded with NO q exchange barrier; q-proj
  rowchunk g feeds attention query-tile group g in a pipeline.
  Attention exploits causality: query tile t (128 rows) only computes
  k-blocks 0..t -- 136 of 256 blocks, identical work on every core.
  attn-out is AllToAll'd so core j does the output projection for its
  own 256 rows over all heads.

The reference masks with 1e-10 (not -inf), so masked entries CAN
dominate softmax when all visible logits are < ~0.  Only possible in
query tile 0 (rows 0..127): handled exactly via an analytic tail:
Z += 1920*exp(1e-10 - M), out += exp(1e-10 - M)/Z * sum(v[128:]).
For rows >= 128 the visible max is >> 37 with overwhelming
probability, so the tail underflows to exactly 0 in fp32, matching
the reference bit-for-bit behaviour.

Precision: logits (std ~2000) are argmax-sharp, so q/k projections and
q@k^T run as 3-pass bf16 hi/lo splits (error ~2^-17 rel).  wq/wk are
pre-split into bf16 hi/lo on the host; pos_bias, wo, v are bf16.
"""

import os

import numpy as np
import ml_dtypes

import concourse.bass as bass
import concourse.mybir as mybir
import concourse.tile as tile
from concourse import bacc, masks
from concourse.bass_utils import run_bass_kernel_spmd

BF16 = ml_dtypes.bfloat16

SEQ = 2048
DIM = 2048
H = 16
DH = 128
P = 128
N_CORES = 8
HPC = 2                  # heads per core: {j, j+8}
MQ = SEQ // N_CORES      # 256 own rows (norm + k/v projections)
NQT = MQ // P            # 2 row tiles of own rows
CD = DIM // P            # 16 contraction chunks
NT = SEQ // P            # 16 seq tiles
GT = 4                   # query tiles per attention group / q rowchunk
NG = NT // GT            # 4 groups
RW = GT * P              # 512 rows per rowchunk
SCALE = DH ** -0.5
EPS = 1e-5
MASKV = 1e-10
NTAIL = float(SEQ - P)   # masked entries beyond tile 0 for rows 0..127

FP = mybir.dt.float32
BF = mybir.dt.bfloat16
U8 = mybir.dt.uint8
AF = mybir.ActivationFunctionType
ALU = mybir.AluOpType
AX = mybir.AxisListType

last_exec_time_ns = None


def _rms_scale_rows(nc, pool, xt, tag):
    """In-place x *= rsqrt(mean(x^2)+eps) for a [P, DIM] tile."""
    sq = pool.tile([P, DIM], FP, tag="sq_scratch", name="sq_scratch", bufs=1)
    ssq = pool.tile([P, 1], FP, tag=f"ssq{tag}", name=f"ssq{tag}")
    nc.scalar.activation(sq[:], xt[:], AF.Square, accum_out=ssq[:])
    nc.vector.tensor_scalar(ssq[:], ssq[:], 1.0 / DIM, EPS, ALU.mult, ALU.add)
    nc.scalar.sqrt(ssq[:], ssq[:])
    nc.vector.reciprocal(ssq[:], ssq[:])
    nc.vector.tensor_scalar_mul(xt[:], xt[:], ssq[:])


def build():
    nc = bacc.Bacc("TRN2", target_bir_lowering=False, debug=False,
                   num_devices=N_CORES)
    xq_d = nc.dram_tensor("xq", [MQ, DIM], FP, kind="ExternalInput")
    g_d = nc.dram_tensor("gamma_t", [P, CD], FP, kind="ExternalInput")
    # per-core slice of wq: my 2 heads, bf16 hi/lo
    wqsh_d = nc.dram_tensor("wqsh", [DIM, HPC * DH], BF, kind="ExternalInput")
    wqsl_d = nc.dram_tensor("wqsl", [DIM, HPC * DH], BF, kind="ExternalInput")
    wkh_d = nc.dram_tensor("wkh", [DIM, DH], BF, kind="ExternalInput")
    wkl_d = nc.dram_tensor("wkl", [DIM, DH], BF, kind="ExternalInput")
    wv_d = nc.dram_tensor("wv", [DIM, DH], BF, kind="ExternalInput")
    wo_d = nc.dram_tensor("wo", [H * DH, DIM], BF, kind="ExternalInput")
    pb_d = nc.dram_tensor("pb", [HPC * SEQ, SEQ], BF, kind="ExternalInput")
    minv_d = nc.dram_tensor("minv", [P, P], U8, kind="ExternalInput")
    out_d = nc.dram_tensor("out", [MQ, DIM], FP, kind="ExternalOutput")

    rg = [list(range(N_CORES))]

    with tile.TileContext(nc) as tc, \
         tc.tile_pool(name="singles", bufs=1) as singles:
        # ---- persistent tiles --------------------------------------------
        ident = singles.tile([P, P], FP, tag="ident", name="ident")
        masks.make_identity(nc, ident[:])
        identb = singles.tile([P, P], BF, tag="identb", name="identb")
        masks.make_identity(nc, identb[:])
        gam = singles.tile([P, CD], FP, tag="gam", name="gam")
        nc.sync.dma_start(out=gam[:], in_=g_d[:])
        minv = singles.tile([P, P], U8, tag="minv", name="minv")
        nc.sync.dma_start(out=minv[:], in_=minv_d[:])
        cfill = singles.tile([P, P], FP, tag="cfill", name="cfill")
        nc.gpsimd.memset(cfill[:], MASKV)
        onesb = singles.tile([P, 1], BF, tag="onesb", name="onesb")
        nc.gpsimd.memset(onesb[:], 1.0)
        # wq slice for my 2 heads, resident in SBUF (loaded after phase 0
        # so the x rows + xn bounce win the DMA rings first)
        wqsh = singles.tile([P, CD, HPC, DH], BF, tag="wqsh", name="wqsh")
        wqsl = singles.tile([P, CD, HPC, DH], BF, tag="wqsl", name="wqsl")

        kTh = singles.tile([P, SEQ], BF, tag="kTh", name="kTh")
        kTl = singles.tile([P, SEQ], BF, tag="kTl", name="kTl")
        vsb = singles.tile([P, NT, DH], BF, tag="vsb", name="vsb")
        S_sb = singles.tile([1, DH], BF, tag="S_sb", name="S_sb")
        # my 2 heads (slots {j, j+8}), all rows
        qTmh = singles.tile([P, HPC, SEQ], BF, tag="qTmh", name="qTmh")
        qTml = singles.tile([P, HPC, SEQ], BF, tag="qTml", name="qTml")
        # my heads' attention output (dh-major)
        oTex = singles.tile([P, HPC, SEQ], BF, tag="oTex", name="oTex")
        # my rows, all heads, post-exchange
        oTfin = singles.tile([P, H, MQ], BF, tag="oTfin", name="oTfin")

        with tc.tile_pool(name="dram", bufs=1, space="DRAM") as dramp:
            # ONE packed AllGather: row-blocks of [128, MQ] so reads are
            # contiguous. rows [c*P,(c+1)*P) = xn hi chunk c; +2048 = lo;
            # 4096:4224 = k hi; 4224:4352 = k lo; 4352:4480 = v (2 blocks
            # side by side in cols)
            AGR = 2 * CD * P + 3 * P              # 4480 rows per rank
            ag_bounce = dramp.tile([AGR, MQ], BF, tag="agb",
                                   name="ag_bounce")
            ag_all = dramp.tile([N_CORES * AGR, MQ], BF, tag="agall",
                                name="ag_all", addr_space="Shared")
            aox = [dramp.tile([N_CORES * P, MQ], BF, tag=f"aox{i}",
                              name=f"aox{i}") for i in range(2)]
            aoxr = [dramp.tile([N_CORES * P, MQ], BF, tag=f"aoxr{i}",
                               name=f"aoxr{i}") for i in range(2)]

            with tc.tile_pool(name="xnp", bufs=1) as xnp:
                xnTh = xnp.tile([P, CD, MQ], BF, tag="xnTh", name="xnTh")
                xnTl = xnp.tile([P, CD, MQ], BF, tag="xnTl", name="xnTl")

                # ---- phase 0: own-row xn^T (hi/lo bf16) ------------------
                with tc.tile_pool(name="ph0", bufs=2) as ph0, \
                     tc.tile_pool(name="pstr0", bufs=2, space="PSUM") as pstr0:
                    xnq = []
                    for t in range(NQT):
                        xt = ph0.tile([P, DIM], FP, tag=f"xq{t}",
                                      name=f"xq{t}")
                        nc.sync.dma_start(out=xt[:],
                                          in_=xq_d[t * P:(t + 1) * P, :])
                        _rms_scale_rows(nc, ph0, xt, f"q{t}")
                        xnq.append(xt)
                    for c in range(CD):
                        pt = pstr0.tile([P, MQ], FP, tag="trq", name="trq")
                        for t in range(NQT):
                            nc.tensor.transpose(pt[:, t * P:(t + 1) * P],
                                                xnq[t][:, c * P:(c + 1) * P],
                                                ident[:])
                        xf = ph0.tile([P, MQ], FP, tag="xnf", name="xnf",
                                      bufs=2)
                        nc.vector.tensor_scalar_mul(xf[:], pt[:],
                                                    gam[:, c:c + 1])
                        nc.vector.tensor_copy(xnTh[:, c, :], xf[:])
                        nc.vector.tensor_tensor(xnTl[:, c, :], xf[:],
                                                xnTh[:, c, :],
                                                op=ALU.subtract)
                # stage xn hi/lo into the packed bounce as produced
                for c in range(CD):
                    nc.sync.dma_start(out=ag_bounce[c * P:(c + 1) * P, :],
                                      in_=xnTh[:, c, :])
                    nc.sync.dma_start(
                        out=ag_bounce[(CD + c) * P:(CD + c + 1) * P, :],
                        in_=xnTl[:, c, :])

                # ---- phase 1: k/v projection + AllGather -----------------
                with tc.tile_pool(name="kvw", bufs=1) as kvwp, \
                     tc.tile_pool(name="psk", bufs=1, space="PSUM") as psk, \
                     tc.tile_pool(name="psv", bufs=1, space="PSUM") as psv, \
                     tc.tile_pool(name="pstv", bufs=2, space="PSUM") as pstv:
                    wkh_sb = kvwp.tile([P, CD, DH], BF, tag="wkh",
                                       name="wkh_sb")
                    wkl_sb = kvwp.tile([P, CD, DH], BF, tag="wkl",
                                       name="wkl_sb")
                    wv_sb = kvwp.tile([P, CD, DH], BF, tag="wv", name="wv_sb")
                    for c in range(CD):
                        nc.sync.dma_start(out=wkh_sb[:, c, :],
                                          in_=wkh_d[c * P:(c + 1) * P, :])
                        nc.sync.dma_start(out=wkl_sb[:, c, :],
                                          in_=wkl_d[c * P:(c + 1) * P, :])
                        nc.sync.dma_start(out=wv_sb[:, c, :],
                                          in_=wv_d[c * P:(c + 1) * P, :])
                    # k^T own rows, 3-pass hi/lo -> fp32
                    pk = psk.tile([P, MQ], FP, tag="pk", name="pk")
                    for c in range(CD):
                        nc.tensor.matmul(pk[:], lhsT=wkh_sb[:, c, :],
                                         rhs=xnTh[:, c, :],
                                         start=(c == 0), stop=False)
                        nc.tensor.matmul(pk[:], lhsT=wkh_sb[:, c, :],
                                         rhs=xnTl[:, c, :],
                                         start=False, stop=False)
                        nc.tensor.matmul(pk[:], lhsT=wkl_sb[:, c, :],
                                         rhs=xnTh[:, c, :],
                                         start=False, stop=(c == CD - 1))
                    kown = kvwp.tile([P, MQ], FP, tag="kown", name="kown")
                    nc.scalar.copy(kown[:], pk[:])
                    kown_h = kvwp.tile([P, MQ], BF, tag="kownh",
                                       name="kown_h")
                    kown_l = kvwp.tile([P, MQ], BF, tag="kownl",
                                       name="kown_l")
                    nc.gpsimd.tensor_copy(kown_h[:], kown[:])
                    nc.gpsimd.tensor_tensor(kown_l[:], kown[:], kown_h[:],
                                            op=ALU.subtract)
                    # v^T own rows (bf16), transpose to [seq, dh]
                    pv = psv.tile([P, MQ], FP, tag="pv", name="pv")
                    for c in range(CD):
                        nc.tensor.matmul(pv[:], lhsT=wv_sb[:, c, :],
                                         rhs=xnTh[:, c, :],
                                         start=(c == 0), stop=(c == CD - 1))
                    vTs = kvwp.tile([P, MQ], FP, tag="vTs", name="vTs")
                    nc.vector.tensor_copy(vTs[:], pv[:])
                    vown = kvwp.tile([P, NQT, DH], BF, tag="vown",
                                     name="vown")
                    for t in range(NQT):
                        ptv = pstv.tile([P, P], FP, tag="vtr", name="vtr")
                        nc.tensor.transpose(ptv[:], vTs[:, t * P:(t + 1) * P],
                                            ident[:])
                        nc.vector.tensor_copy(vown[:, t, :], ptv[:])
                    # stage k hi/lo + v into the packed bounce, ONE AllGather
                    KO = 2 * CD * P
                    nc.sync.dma_start(out=ag_bounce[KO:KO + P, :],
                                      in_=kown_h[:])
                    nc.sync.dma_start(out=ag_bounce[KO + P:KO + 2 * P, :],
                                      in_=kown_l[:])
                    for t in range(NQT):
                        nc.sync.dma_start(
                            out=ag_bounce[KO + 2 * P:KO + 3 * P,
                                          t * DH:(t + 1) * DH],
                            in_=vown[:, t, :])
                    nc.gpsimd.collective_compute(
                        "AllGather", ALU.bypass, replica_groups=rg,
                        ins=[ag_bounce[:].opt()], outs=[ag_all[:].opt()])
                    for r in range(N_CORES):
                        base = r * AGR + KO
                        nc.scalar.dma_start(
                            out=kTh[:, r * MQ:(r + 1) * MQ],
                            in_=ag_all[base:base + P, :])
                        nc.scalar.dma_start(
                            out=kTl[:, r * MQ:(r + 1) * MQ],
                            in_=ag_all[base + P:base + 2 * P, :])
                        for t in range(NQT):
                            nc.scalar.dma_start(
                                out=vsb[:, 2 * r + t, :],
                                in_=ag_all[base + 2 * P:base + 3 * P,
                                           t * DH:(t + 1) * DH])
                    for c in range(CD):
                        nc.sync.dma_start(out=wqsh[:, c, :, :],
                                          in_=wqsh_d[c * P:(c + 1) * P, :])
                        nc.sync.dma_start(out=wqsl[:, c, :, :],
                                          in_=wqsl_d[c * P:(c + 1) * P, :])

            # ---- phase 2+3: q proj (from xn_ag) pipelined w/ attention ---
            with tc.tile_pool(name="qxp", bufs=3) as qxp, \
                 tc.tile_pool(name="qst", bufs=2) as qstp, \
                 tc.tile_pool(name="att", bufs=1) as att, \
                 tc.tile_pool(name="pos", bufs=5) as posp, \
                 tc.tile_pool(name="simp", bufs=4) as simp, \
                 tc.tile_pool(name="pexpp", bufs=4) as pexpp, \
                 tc.tile_pool(name="ptp", bufs=2) as ptp, \
                 tc.tile_pool(name="stp", bufs=8) as stp, \
                 tc.tile_pool(name="psq", bufs=1, space="PSUM") as psq, \
                 tc.tile_pool(name="ps_sim", bufs=2, space="PSUM") as ps_sim, \
                 tc.tile_pool(name="ps_pt", bufs=2, space="PSUM") as ps_pt, \
                 tc.tile_pool(name="ps_o", bufs=1, space="PSUM") as ps_o:
                # the [P,1]->[1,P] crec transpose borrows a ps_sim tile
                ps_ct = ps_sim

                def qproj_rc(rc):
                    """q^T for both my heads, rows [512rc, 512rc+512)."""
                    pqs = [psq.tile([P, RW], FP, tag=f"pq{s}", name=f"pq{s}")
                           for s in range(HPC)]
                    for c in range(CD):
                        xh = qxp.tile([P, RW], BF, tag="xh", name="xh")
                        xl = qxp.tile([P, RW], BF, tag="xl", name="xl")
                        for half in range(2):
                            r = 2 * rc + half
                            base = r * AGR
                            hs = slice(base + c * P, base + (c + 1) * P)
                            ls = slice(base + (CD + c) * P,
                                       base + (CD + c + 1) * P)
                            ds = slice(half * MQ, (half + 1) * MQ)
                            nc.scalar.dma_start(out=xh[:, ds],
                                                in_=ag_all[hs, :])
                            nc.scalar.dma_start(out=xl[:, ds],
                                                in_=ag_all[ls, :])
                        for s in range(HPC):
                            nc.tensor.matmul(pqs[s][:],
                                             lhsT=wqsh[:, c, s, :],
                                             rhs=xh[:],
                                             start=(c == 0), stop=False)
                            nc.tensor.matmul(pqs[s][:],
                                             lhsT=wqsh[:, c, s, :],
                                             rhs=xl[:],
                                             start=False, stop=False)
                            nc.tensor.matmul(pqs[s][:],
                                             lhsT=wqsl[:, c, s, :],
                                             rhs=xh[:],
                                             start=False,
                                             stop=(c == CD - 1))
                    rs = slice(rc * RW, (rc + 1) * RW)
                    for s in range(HPC):
                        qs = qstp.tile([P, RW], FP, tag="qs", name="qs")
                        nc.vector.tensor_scalar_mul(qs[:], pqs[s][:], SCALE)
                        nc.gpsimd.tensor_copy(qTmh[:, s, rs], qs[:])
                        nc.gpsimd.tensor_tensor(qTml[:, s, rs], qs[:],
                                                qTmh[:, s, rs],
                                                op=ALU.subtract)

                def softmax_tile(slot, t, crec_out):
                    """sim+softmax for query tile t; returns pexp tile."""
                    nk = t + 1
                    w = nk * P
                    pos_t = posp.tile([P, SEQ], BF, tag="pos", name="pos")
                    nc.sync.dma_start(
                        out=pos_t[:, :w],
                        in_=pb_d[slot * SEQ + t * P:slot * SEQ + (t + 1) * P,
                                 0:w])
                    sim = simp.tile([P, SEQ], FP, tag="sim", name="sim")
                    qsl = slice(t * P, (t + 1) * P)
                    # chunk-pair the sim matmuls so each qh/ql LDWEIGHTS
                    # covers two 512-wide chunks
                    for c0 in range(0, w, 1024):
                        cws = []
                        psims = []
                        for cc in (c0, c0 + 512):
                            if cc < w:
                                cw = min(512, w - cc)
                                cws.append((cc, cw))
                                psims.append(ps_sim.tile([P, 512], FP,
                                                         tag="psim",
                                                         name="psim"))
                        for i, (cc, cw) in enumerate(cws):
                            nc.tensor.matmul(psims[i][:, :cw],
                                             lhsT=qTmh[:, slot, qsl],
                                             rhs=kTh[:, cc:cc + cw],
                                             start=True, stop=False)
                            nc.tensor.matmul(psims[i][:, :cw],
                                             lhsT=qTmh[:, slot, qsl],
                                             rhs=kTl[:, cc:cc + cw],
                                             start=False, stop=False)
                        for i, (cc, cw) in enumerate(cws):
                            nc.tensor.matmul(psims[i][:, :cw],
                                             lhsT=qTml[:, slot, qsl],
                                             rhs=kTh[:, cc:cc + cw],
                                             start=False, stop=True)
                        for i, (cc, cw) in enumerate(cws):
                            nc.vector.tensor_tensor(sim[:, cc:cc + cw],
                                                    psims[i][:, :cw],
                                                    pos_t[:, cc:cc + cw],
                                                    op=ALU.add)
                    # mask the diagonal block (last 128 cols)
                    nc.vector.copy_predicated(sim[:, w - P:w], minv[:],
                                              cfill[:])
                    negmax = stp.tile([P, 1], FP, tag="negmax",
                                      name="negmax")
                    nc.vector.tensor_reduce(negmax[:], sim[:, :w], axis=AX.X,
                                            op=ALU.max, negate=True)
                    if t == 0:
                        # M = max(visible, 1e-10): negmax = min(negmax, -1e-10)
                        nc.vector.tensor_scalar(negmax[:], negmax[:], 1.0,
                                                -MASKV, ALU.mult, ALU.min)
                    pexp = pexpp.tile([P, SEQ], BF, tag="pexp", name="pexp")
                    ssum = stp.tile([P, 1], FP, tag="ssum", name="ssum")
                    nc.scalar.activation(pexp[:, :w], sim[:, :w], AF.Exp,
                                         bias=negmax[:], accum_out=ssum[:])
                    rec = stp.tile([P, 1], FP, tag="rec", name="rec")
                    if t == 0:
                        # Z += 1920 * exp(1e-10 - M); tail folded into attn@v
                        ce = stp.tile([P, 1], FP, tag="ce", name="ce")
                        nc.scalar.activation(ce[:], negmax[:], AF.Exp)
                        ct = stp.tile([P, 1], FP, tag="ct", name="ct")
                        nc.vector.tensor_scalar_mul(ct[:], ce[:], NTAIL)
                        nc.vector.tensor_tensor(ssum[:], ssum[:], ct[:],
                                                op=ALU.add)
                        nc.vector.reciprocal(rec[:], ssum[:])
                        crec = stp.tile([P, 1], FP, tag="crec", name="crec")
                        nc.vector.tensor_tensor(crec[:], ce[:], rec[:],
                                                op=ALU.mult)
                        # transpose [P,1] -> [1,P] via a padded [P,P]
                        cpad = stp.tile([P, P], FP, tag="cpad", name="cpad",
                                        bufs=1)
                        nc.gpsimd.memset(cpad[:], 0.0)
                        nc.vector.tensor_copy(cpad[:, 0:1], crec[:])
                        pct = ps_ct.tile([P, 512], FP, tag="psim",
                                         name="psim")
                        nc.tensor.transpose(pct[:, 0:P], cpad[:], ident[:])
                        nc.scalar.copy(crec_out[:], pct[0:1, 0:P])
                    else:
                        nc.vector.reciprocal(rec[:], ssum[:])
                    nc.vector.tensor_scalar_mul(pexp[:, :w], pexp[:, :w],
                                                rec[:])
                    return pexp

                def attn_group(slot, gr, crecT):
                    PT = ptp.tile([P, NT, GT, P], BF, tag="PT", name="PT")
                    # zero-pad slots (s, a) with 4*gr + a < s
                    for k in range(1, GT):
                        s = 4 * gr + k
                        nc.gpsimd.memset(PT[:, s, 0:k, :], 0.0)
                    pexps = []
                    for a in range(GT):
                        pexps.append(softmax_tile(slot, 4 * gr + a, crecT))
                    for a in range(GT):
                        t = 4 * gr + a
                        nk = t + 1
                        pexp = pexps[a]
                        for s0 in range(0, nk, 4):
                            sz = min(4, nk - s0)
                            ppt = ps_pt.tile([P, 4 * P], BF, tag="ppt",
                                             name="ppt")
                            for s4 in range(sz):
                                nc.tensor.transpose(
                                    ppt[:, s4 * P:(s4 + 1) * P],
                                    pexp[:, (s0 + s4) * P:(s0 + s4 + 1) * P],
                                    identb[:])
                            nc.scalar.copy(PT[:, s0:s0 + sz, a, :],
                                           ppt[:, :sz * P])
                    # attn@v for the group (padded full-width rhs)
                    po = ps_o.tile([P, GT * P], FP, tag="po", name="po")
                    last = 4 * gr + GT - 1
                    for s in range(0, last + 1):
                        nc.tensor.matmul(po[:], lhsT=vsb[:, s, :],
                                         rhs=PT[:, s, :, :],
                                         start=(s == 0), stop=(s == last))
                        if gr == 0 and s == 0:
                            # analytic masked tail for rows 0..127
                            nc.tensor.matmul(po[:, 0:P], lhsT=S_sb[:],
                                             rhs=crecT[:],
                                             start=False, stop=False)
                    nc.vector.tensor_copy(
                        oTex[:, slot, gr * RW:(gr + 1) * RW], po[:])

                def ao_exchange(slot):
                    # ship this slot's attn-out: section m = rank m's rows
                    for m in range(N_CORES):
                        nc.sync.dma_start(
                            out=aox[slot][m * P:(m + 1) * P, :],
                            in_=oTex[:, slot, m * MQ:(m + 1) * MQ])
                    nc.gpsimd.collective_compute(
                        "AllToAll", ALU.bypass, replica_groups=rg,
                        ins=[aox[slot][:].opt()], outs=[aoxr[slot][:].opt()])
                    # section r = head r + 8*slot over my rows
                    for r in range(N_CORES):
                        nc.scalar.dma_start(
                            out=oTfin[:, r + N_CORES * slot, :],
                            in_=aoxr[slot][r * P:(r + 1) * P, :])

                crecT = [att.tile([1, P], BF, tag=f"crecT{s}",
                                  name=f"crecT{s}") for s in range(HPC)]
                qproj_rc(0)
                qproj_rc(1)
                # S = sum of v rows 128..2047 (tail for tile 0) -- emitted
                # here so it does not block the PE queue before qproj
                ps_s = psq.tile([1, DH], FP, tag="ps_s", name="ps_s")
                for s in range(1, NT):
                    nc.tensor.matmul(ps_s[:], lhsT=onesb[:],
                                     rhs=vsb[:, s, :],
                                     start=(s == 1), stop=(s == NT - 1))
                nc.scalar.copy(S_sb[:], ps_s[:])
                attn_group(0, 0, crecT[0])
                attn_group(1, 0, crecT[1])
                qproj_rc(2)
                attn_group(0, 1, crecT[0])
                attn_group(1, 1, crecT[1])
                qproj_rc(3)
                attn_group(0, 2, crecT[0])
                attn_group(1, 2, crecT[1])
                attn_group(0, 3, crecT[0])
                ao_exchange(0)
                attn_group(1, 3, crecT[1])
                ao_exchange(1)

        # ---- phase 4: output projection (bf16) ---------------------------
        with tc.tile_pool(name="wop", bufs=3) as wop, \
             tc.tile_pool(name="osb", bufs=2) as osbp, \
             tc.tile_pool(name="ps_out", bufs=NQT * (DIM // 512),
                          space="PSUM") as ps_out:
            pouts = []
            for t in range(NQT):
                for nk in range(DIM // 512):
                    pouts.append(ps_out.tile([P, 512], FP, tag="pout",
                                             name=f"pout{t}_{nk}"))
            for h in range(H):
                wo_b = wop.tile([P, DIM], BF, tag="wob", name="wo_b")
                nc.sync.dma_start(out=wo_b[:],
                                  in_=wo_d[h * DH:(h + 1) * DH, :])
                for t in range(NQT):
                    for nk in range(DIM // 512):
                        nc.tensor.matmul(pouts[t * (DIM // 512) + nk][:],
                                         lhsT=oTfin[:, h, t * P:(t + 1) * P],
                                         rhs=wo_b[:, nk * 512:(nk + 1) * 512],
                                         start=(h == 0), stop=(h == H - 1))
            for t in range(NQT):
                osb = osbp.tile([P, DIM], FP, tag="osb", name="osb")
                for nk in range(DIM // 512):
                    nc.scalar.copy(osb[:, nk * 512:(nk + 1) * 512],
                                   pouts[t * (DIM // 512) + nk][:])
                    nc.sync.dma_start(
                        out=out_d[t * P:(t + 1) * P, nk * 512:(nk + 1) * 512],
                        in_=osb[:, nk * 512:(nk + 1) * 512])

    nc.compile()
    return nc


_NC = None


def kernel(**inputs):
    global _NC, last_exec_time_ns
    x = np.asarray(inputs["x"], dtype=np.float32)[0]          # [SEQ, DIM]
    pos = np.asarray(inputs["pos_bias"], dtype=np.float32)    # [H, SEQ, SEQ]
    gamma = np.asarray(inputs["gamma"], dtype=np.float32)
    wq = np.asarray(inputs["wq"], dtype=np.float32)
    wk = np.asarray(inputs["wk"], dtype=np.float32)
    wv = np.asarray(inputs["wv"], dtype=np.float32)
    wo = np.asarray(inputs["wo"], dtype=np.float32)

    if _NC is None:
        _NC = build()

    def hilo(w):
        hi = w.astype(BF16)
        lo = (w - hi.astype(np.float32)).astype(BF16)
        return np.ascontiguousarray(hi), np.ascontiguousarray(lo)

    wk_hi, wk_lo = hilo(wk)
    wv_b = np.ascontiguousarray(wv.astype(BF16))
    wo_b = np.ascontiguousarray(wo.astype(BF16))
    pos_b = pos.astype(BF16)
    gamma_t = np.ascontiguousarray(gamma.reshape(CD, P).T)
    # strict upper triangle = masked-out within the diagonal block
    minv = np.ascontiguousarray(
        np.triu(np.ones((P, P), dtype=np.uint8), k=1))

    in_maps = []
    for m in range(N_CORES):
        q0 = m * MQ
        wqs = np.concatenate(
            [wq[:, m * DH:(m + 1) * DH],
             wq[:, (m + N_CORES) * DH:(m + N_CORES + 1) * DH]], axis=1)
        wqs_hi, wqs_lo = hilo(wqs)
        in_maps.append({
            "xq": np.ascontiguousarray(x[q0:q0 + MQ]),
            "gamma_t": gamma_t,
            "wqsh": wqs_hi, "wqsl": wqs_lo,
            "wkh": wk_hi, "wkl": wk_lo,
            "wv": wv_b, "wo": wo_b,
            "pb": np.ascontiguousarray(
                np.concatenate([pos_b[m], pos_b[m + N_CORES]], axis=0)),
            "minv": minv,
        })
    trace = os.environ.get("KERNEL_TRACE") == "1"
    res = run_bass_kernel_spmd(_NC, in_maps, core_ids=list(range(N_CORES)),
                               trace=trace)
    last_exec_time_ns = res.exec_time_ns
    out = np.concatenate([res.results[m]["out"] for m in range(N_CORES)],
                         axis=0)[None, ...]
    return out.astype(np.float32)
